# revision 23
# baseline (speedup 1.0000x reference)
"""DeChunkLayer Trainium2 kernel.

Computation (per batch row b):
    proj = x[b] @ W.T + b                                  # [C, D]
    nproj = LayerNorm(proj) (= (proj-mean)*rsqrt(var+eps)*gamma + beta)
    out[t] = nproj[idx[t]] for valid t, beta for padded t  # [L, D]

Key insight: LayerNorm commutes with the ragged expansion (each output row is a
copy of a chunk row, so its stats equal the chunk row's stats). So we normalize
the C=2048 chunk rows once and then expand, instead of normalizing L=8192 rows.

The ragged expansion runs on the TensorEngine as a block-sparse one-hot matmul:
for each 128-token output block, accumulate over the few 128-chunk tiles the
block draws from:  out_block = sum_ct OH[k,ct].T @ nproj[ct].  The one-hot
pieces are built ON DEVICE from a tiny relative-index vector (host-computed from
chunk_lengths): a K=1 matmul broadcasts rel-indices across partitions into
PSUM, then one DVE is_equal against a partition iota writes the bf16 one-hot.
Piece->tile wiring is baked at trace time (kernel JIT-specialized per input,
SPMD-uniform across cores: piece list is the union over cores; per-core rel
indices land outside [0,128) wherever a core doesn't use a piece, giving zero
one-hot columns there).

Sharding: data-parallel over the batch dim, one NeuronCore per row (8 cores);
W/bias/gamma/beta replicated. Padded tail positions map to a dedicated beta row
(row C) of the nproj table.
"""

import sys

for _p in ("/opt/trn_rl_repo",):
    if _p not in sys.path:
        sys.path.insert(0, _p)

import numpy as np
import ml_dtypes

import concourse.bass as bass
import concourse.tile as tile
from concourse import bacc, mybir
from concourse.bass_utils import run_bass_kernel_spmd

B, C, D, L = 8, 2048, 512, 8192
P = 128           # partitions
CT = C // P       # 16 chunk tiles
NT = L // P       # 64 output-token blocks
KT = D // P       # 4 contraction tiles
EPS = 1e-5
GRP = 8           # output blocks per store DMA (2 MiB batches)

F32 = mybir.dt.float32
BF16 = mybir.dt.bfloat16
F16 = mybir.dt.float16
I32 = mybir.dt.int32
BF = ml_dtypes.bfloat16


# ----------------------------------------------------------------- host logic

def _build_indices(chunk_lengths: np.ndarray) -> np.ndarray:
    """idx[b, t] = source chunk row for output token t; C means 'beta row'."""
    cum = np.cumsum(chunk_lengths.astype(np.int64), axis=1)
    pos = np.arange(L)
    idx = np.empty((B, L), np.int32)
    for b in range(B):
        i = np.searchsorted(cum[b], pos, side="right").astype(np.int32)
        i = np.minimum(i, C - 1)
        i[pos >= cum[b, -1]] = C  # padded positions -> beta row
        idx[b] = i
    return idx


def _build_pieces(idx: np.ndarray):
    """Per 128-token block, the union (over cores) of touched 128-chunk tiles."""
    blocks = idx.reshape(B, NT, P)
    piece_meta = []  # list of (block k, chunk-tile ct)
    for k in range(NT):
        tiles = sorted(set((blocks[:, k, :] // P).ravel().tolist()))
        for ct in tiles:
            piece_meta.append((k, ct))
    return piece_meta


def _build_relidx(idx: np.ndarray, piece_meta) -> np.ndarray:
    """relidx[b, 0, 128p+tt] = idx[b, 128k+tt] - 128*ct for piece p=(k,ct).

    On device this is partition-broadcast and compared to the partition iota j:
    one-hot[j, col] = (relidx[col] == j). Values outside [0,128) never match.
    """
    NP = len(piece_meta)
    blocks = idx.reshape(B, NT, P).astype(np.float32)
    rel = np.empty((B, 1, NP * P), np.float16)  # integers <=2048: exact in fp16
    for p, (k, ct) in enumerate(piece_meta):
        rel[:, 0, P * p:P * (p + 1)] = blocks[:, k, :] - P * ct
    return rel


# --------------------------------------------------------------- bass kernel

def _build_bass(piece_meta, has_b: bool, trivial_affine: bool, has_beta_row: bool):
    NP = len(piece_meta)
    NPC = NP * P
    # Bacc (not bare Bass): its compile() legalizes drains/sync-waits into
    # forms the walrus codegen accepts.
    nc = bacc.Bacc(None)

    xT = nc.declare_dram_parameter("xT", [D, C], F16, isOutput=False)
    wt = nc.declare_dram_parameter("wt", [D, D], F16, isOutput=False)
    relidx = nc.declare_dram_parameter("relidx", [1, NPC], F16, isOutput=False)
    if has_b:
        bvec = nc.declare_dram_parameter("bvec", [1, D], F16, isOutput=False)
    if not trivial_affine:
        gamma = nc.declare_dram_parameter("gamma", [1, D], F32, isOutput=False)
        beta = nc.declare_dram_parameter("beta", [1, D], F32, isOutput=False)
    out = nc.declare_dram_parameter("out", [L, D], F32, isOutput=True)

    with tile.TileContext(nc) as tc:
        with (
            tc.tile_pool(name="singles", bufs=1) as singles,
            tc.tile_pool(name="stats", bufs=8) as stats_pool,
            tc.tile_pool(name="ostage_a", bufs=2) as ostage_a,
            tc.tile_pool(name="ostage_v", bufs=2) as ostage_v,
            tc.tile_pool(name="psA", bufs=2, space="PSUM") as psA,
            tc.tile_pool(name="psB", bufs=6, space="PSUM") as psB,
        ):
            # ---- tiny consts
            eps_sb = singles.tile([P, 1], F32)
            nc.vector.memset(eps_sb, EPS)
            iota_i = singles.tile([P, 1], I32)
            nc.gpsimd.iota(iota_i, pattern=[[0, 1]], base=0, channel_multiplier=1)
            iota_f = singles.tile([P, 1], F32)
            nc.vector.tensor_copy(out=iota_f, in_=iota_i)

            # ---- input DMAs (host provides layouts that DMA at line rate)
            relidx_sb = singles.tile([1, NPC], F16)
            nc.sync.dma_start(out=relidx_sb, in_=relidx[:, :])
            xT_sb = singles.tile([P, KT, C], F16)
            H = C // 2
            for kt in range(KT):
                nc.sync.dma_start(
                    out=xT_sb[:, kt, 0:H],
                    in_=xT[P * kt:P * (kt + 1), 0:H],
                )
            wt_sb = singles.tile([P, KT, D], F16)
            nc.sync.dma_start(
                out=wt_sb, in_=wt.rearrange("(kt p) e -> p kt e", p=P)
            )
            for kt in range(KT):
                nc.sync.dma_start(
                    out=xT_sb[:, kt, H:C],
                    in_=xT[P * kt:P * (kt + 1), H:C],
                )

            if has_b:
                onesb_sb = singles.tile([1, P], F16)
                nc.vector.memset(onesb_sb, 1.0)
                bvec_sb = singles.tile([1, D], F16)
                nc.sync.dma_start(out=bvec_sb, in_=bvec[:, :])
            if not trivial_affine:
                gamma_sb = singles.tile([P, D], F32)
                nc.gpsimd.dma_start(out=gamma_sb, in_=gamma.to_broadcast((P, D)))
                beta_sb = singles.tile([P, D], F32)
                nc.gpsimd.dma_start(out=beta_sb, in_=beta.to_broadcast((P, D)))

            # normalized projection table; tile CT holds the beta row (row 0)
            nproj = singles.tile([P, CT + 1, D], F16)
            if has_beta_row:
                nc.vector.memset(nproj[:, CT, :], 0.0)
                if not trivial_affine:
                    nc.vector.tensor_copy(out=nproj[0:1, CT, :], in_=beta_sb[0:1, :])

            # ---- build one-hot pieces on device:
            # GPSIMD broadcasts rel-indices across partitions (PE stays free
            # for matmuls), then DVE is_equal against the partition iota
            # (16-bit SBUF operands -> DVE fast mode) writes the bf16 one-hot.
            relb = singles.tile([P, NPC], F16)
            oh_sb = singles.tile([P, NPC], F16)
            NCH = 8
            chunk = ((NPC // NCH) // P) * P
            bounds = [i * chunk for i in range(NCH)] + [NPC]
            for i in range(NCH):
                c0, c1 = bounds[i], bounds[i + 1]
                nc.gpsimd.partition_broadcast(
                    relb[:, c0:c1], relidx_sb[0:1, c0:c1]
                )
                eng = nc.vector if i % 2 == 0 else nc.gpsimd
                eng.tensor_scalar(
                    out=oh_sb[:, c0:c1], in0=relb[:, c0:c1],
                    scalar1=iota_f, scalar2=None,
                    op0=mybir.AluOpType.is_equal,
                )

            # ---- stage B bookkeeping: store groups emitted as soon as the
            # chunk tiles they read are normalized (overlap stores with stage A)
            by_block = [[] for _ in range(NT)]
            for p, (k, ct) in enumerate(piece_meta):
                by_block[k].append((p, ct))
            ngroups = NT // GRP
            group_need = []
            for g in range(ngroups):
                need = 0
                for k in range(g * GRP, (g + 1) * GRP):
                    for _, ct in by_block[k]:
                        if ct < CT:
                            need = max(need, ct)
                group_need.append(need)

            def emit_group(g: int):
                use_act = g != 3  # 7 groups on ACT, 1 on DVE (load balance)
                ot = (ostage_a if use_act else ostage_v).tile([P, GRP, D], F32)
                # tiny same-engine touch op claims the slot and absorbs the
                # DMA-release wait, keeping the big copies at <=2 sync waits
                # (walrus codegen rejects more)
                if use_act:
                    nc.scalar.copy(out=ot[0:1, 0, 0:1], in_=eps_sb[0:1, 0:1])
                else:
                    nc.vector.memset(ot[0:1, 0, 0:1], 0.0)
                for ki in range(GRP):
                    k = g * GRP + ki
                    pieces = by_block[k]
                    go = psB.tile([P, D], F32)
                    for i, (p, ct) in enumerate(pieces):
                        nc.tensor.matmul(
                            go,
                            lhsT=oh_sb[:, P * p:P * (p + 1)],
                            rhs=nproj[:, ct, :],
                            start=(i == 0),
                            stop=(i == len(pieces) - 1),
                        )
                    if use_act:
                        nc.scalar.copy(out=ot[:, ki, :], in_=go)
                    else:
                        nc.vector.tensor_copy(out=ot[:, ki, :], in_=go)
                nc.sync.dma_start(
                    out=out[P * GRP * g:P * GRP * (g + 1), :].rearrange(
                        "(c p) e -> p c e", p=P
                    ),
                    in_=ot,
                )

            # ---- stage A: proj = x @ W.T (+b) -> LayerNorm -> nproj (bf16),
            # with ready store-groups interleaved
            for ct in range(CT):
                pj = psA.tile([P, D], F32)
                for kt in range(KT):
                    nc.tensor.matmul(
                        pj,
                        lhsT=xT_sb[:, kt, P * ct:P * (ct + 1)],
                        rhs=wt_sb[:, kt, :],
                        start=(kt == 0),
                        stop=(kt == KT - 1 and not has_b),
                    )
                if has_b:
                    nc.tensor.matmul(
                        pj, lhsT=onesb_sb, rhs=bvec_sb, start=False, stop=True,
                    )

                st = stats_pool.tile([P, 6], F32, tag="bn")
                nc.vector.bn_stats(out=st, in_=pj)
                mv = stats_pool.tile([P, 2], F32, tag="mv")
                nc.vector.bn_aggr(out=mv, in_=st)
                rstd = stats_pool.tile([P, 1], F32, tag="rstd")
                nc.scalar.activation(
                    out=rstd, in_=mv[:, 1:2],
                    func=mybir.ActivationFunctionType.Sqrt,
                    bias=eps_sb, scale=1.0,
                )
                nc.vector.reciprocal(out=rstd, in_=rstd)
                if trivial_affine:
                    nc.vector.tensor_scalar(
                        out=nproj[:, ct, :], in0=pj,
                        scalar1=mv[:, 0:1], scalar2=rstd,
                        op0=mybir.AluOpType.subtract, op1=mybir.AluOpType.mult,
                    )
                else:
                    tmp = stats_pool.tile([P, D], F32, tag="affine")
                    nc.vector.tensor_scalar(
                        out=tmp, in0=pj,
                        scalar1=mv[:, 0:1], scalar2=rstd,
                        op0=mybir.AluOpType.subtract, op1=mybir.AluOpType.mult,
                    )
                    nc.vector.tensor_mul(out=tmp, in0=tmp, in1=gamma_sb)
                    nc.vector.tensor_add(out=nproj[:, ct, :], in0=tmp, in1=beta_sb)

                for g in range(ngroups):
                    if group_need[g] == ct:
                        emit_group(g)

    nc.compile()
    return nc


# ---------------------------------------------------------------- entrypoint

def kernel(**inputs) -> np.ndarray:
    return run_dechunk(inputs)[0]


def run_dechunk(inputs, trace=False, tmpdir=None):
    x = np.asarray(inputs["hierarchical_repr"], np.float32)
    W = np.asarray(inputs["W"], np.float32)
    bvec = np.asarray(inputs["b"], np.float32)
    gamma = np.asarray(inputs["gamma"], np.float32)
    beta = np.asarray(inputs["beta"], np.float32)
    cl = np.asarray(inputs["chunk_lengths"]).astype(np.int32)
    seq_len = int(np.asarray(inputs["seq_len"]))
    assert x.shape == (B, C, D) and W.shape == (D, D) and seq_len == L

    idx = _build_indices(cl)
    piece_meta = _build_pieces(idx)
    rel = _build_relidx(idx, piece_meta)

    has_b = bool(np.any(bvec != 0))
    trivial_affine = bool(np.all(gamma == 1.0) and np.all(beta == 0.0))
    has_beta_row = bool((idx == C).any())

    nc = _build_bass(piece_meta, has_b, trivial_affine, has_beta_row)

    wt_h = np.ascontiguousarray(W.T).astype(np.float16)
    in_maps = []
    for b in range(B):
        m = {
            "xT": np.ascontiguousarray(x[b].T).astype(np.float16),
            "wt": wt_h,
            "relidx": rel[b],
        }
        if has_b:
            m["bvec"] = bvec.reshape(1, D).astype(np.float16)
        if not trivial_affine:
            m["gamma"] = gamma.reshape(1, D)
            m["beta"] = beta.reshape(1, D)
        in_maps.append(m)

    res = run_bass_kernel_spmd(
        nc, in_maps, core_ids=list(range(B)), trace=trace, tmpdir=tmpdir
    )
    out = np.stack(
        [np.asarray(res.results[i]["out"], np.float32) for i in range(B)]
    )
    return out, res


# revision 25
# speedup vs baseline: 1.5676x; 1.5676x over previous
"""DeChunkLayer Trainium2 kernel.

Computation (per batch row b):
    proj = x[b] @ W.T + b                                  # [C, D]
    nproj = LayerNorm(proj) (= (proj-mean)*rsqrt(var+eps)*gamma + beta)
    out[t] = nproj[idx[t]] for valid t, beta for padded t  # [L, D]

Key insight: LayerNorm commutes with the ragged expansion (each output row is a
copy of a chunk row, so its stats equal the chunk row's stats). So we normalize
the C=2048 chunk rows once and then expand, instead of normalizing L=8192 rows.

The ragged expansion runs on the TensorEngine as a block-sparse one-hot matmul:
for each 128-token output block, accumulate over the few 128-chunk tiles the
block draws from:  out_block = sum_ct OH[k,ct].T @ nproj[ct].  The one-hot
pieces are built ON DEVICE from a tiny relative-index vector (host-computed from
chunk_lengths): a K=1 matmul broadcasts rel-indices across partitions into
PSUM, then one DVE is_equal against a partition iota writes the bf16 one-hot.
Piece->tile wiring is baked at trace time (kernel JIT-specialized per input,
SPMD-uniform across cores: piece list is the union over cores; per-core rel
indices land outside [0,128) wherever a core doesn't use a piece, giving zero
one-hot columns there).

Sharding: data-parallel over the batch dim, one NeuronCore per row (8 cores);
W/bias/gamma/beta replicated. Padded tail positions map to a dedicated beta row
(row C) of the nproj table.
"""

import sys

for _p in ("/opt/trn_rl_repo",):
    if _p not in sys.path:
        sys.path.insert(0, _p)

import numpy as np
import ml_dtypes

import concourse.bass as bass
import concourse.tile as tile
from concourse import bacc, mybir
from concourse.bass_utils import run_bass_kernel_spmd

B, C, D, L = 8, 2048, 512, 8192
P = 128           # partitions
CT = C // P       # 16 chunk tiles
NT = L // P       # 64 output-token blocks
KT = D // P       # 4 contraction tiles
EPS = 1e-5
GRP = 8           # output blocks per store DMA (2 MiB batches)

F32 = mybir.dt.float32
BF16 = mybir.dt.bfloat16
F16 = mybir.dt.float16
I32 = mybir.dt.int32
BF = ml_dtypes.bfloat16


# ----------------------------------------------------------------- host logic

def _build_indices(chunk_lengths: np.ndarray) -> np.ndarray:
    """idx[b, t] = source chunk row for output token t; C means 'beta row'."""
    cum = np.cumsum(chunk_lengths.astype(np.int64), axis=1)
    pos = np.arange(L)
    idx = np.empty((B, L), np.int32)
    for b in range(B):
        i = np.searchsorted(cum[b], pos, side="right").astype(np.int32)
        i = np.minimum(i, C - 1)
        i[pos >= cum[b, -1]] = C  # padded positions -> beta row
        idx[b] = i
    return idx


def _build_pieces(idx: np.ndarray):
    """Per 128-token block, the union (over cores) of touched 128-chunk tiles."""
    blocks = idx.reshape(B, NT, P)
    piece_meta = []  # list of (block k, chunk-tile ct)
    for k in range(NT):
        tiles = sorted(set((blocks[:, k, :] // P).ravel().tolist()))
        for ct in tiles:
            piece_meta.append((k, ct))
    return piece_meta


def _build_relidx(idx: np.ndarray, piece_meta) -> np.ndarray:
    """relidx[b, 0, 128p+tt] = idx[b, 128k+tt] - 128*ct for piece p=(k,ct).

    On device this is partition-broadcast and compared to the partition iota j:
    one-hot[j, col] = (relidx[col] == j). Values outside [0,128) never match.
    """
    NP = len(piece_meta)
    blocks = idx.reshape(B, NT, P).astype(np.float32)
    rel = np.empty((B, 1, NP * P), np.float16)  # integers <=2048: exact in fp16
    for p, (k, ct) in enumerate(piece_meta):
        rel[:, 0, P * p:P * (p + 1)] = blocks[:, k, :] - P * ct
    return rel


# --------------------------------------------------------------- bass kernel

def _build_bass(piece_meta, has_b: bool, trivial_affine: bool, has_beta_row: bool):
    NP = len(piece_meta)
    NPC = NP * P
    # Bacc (not bare Bass): its compile() legalizes drains/sync-waits into
    # forms the walrus codegen accepts.
    nc = bacc.Bacc(None)

    xT = nc.declare_dram_parameter("xT", [D, C], F16, isOutput=False)
    wt = nc.declare_dram_parameter("wt", [D, D], F16, isOutput=False)
    relidx = nc.declare_dram_parameter("relidx", [1, NPC], F16, isOutput=False)
    if has_b:
        bvec = nc.declare_dram_parameter("bvec", [1, D], F16, isOutput=False)
    if not trivial_affine:
        gamma = nc.declare_dram_parameter("gamma", [1, D], F32, isOutput=False)
        beta = nc.declare_dram_parameter("beta", [1, D], F32, isOutput=False)
    out = nc.declare_dram_parameter("out", [L, D], F32, isOutput=True)

    with tile.TileContext(nc) as tc:
        with (
            tc.tile_pool(name="singles", bufs=1) as singles,
            tc.tile_pool(name="stats", bufs=8) as stats_pool,
            tc.tile_pool(name="ostage_a", bufs=2) as ostage_a,
            tc.tile_pool(name="ostage_v", bufs=2) as ostage_v,
            tc.tile_pool(name="psA", bufs=2, space="PSUM") as psA,
            tc.tile_pool(name="psB", bufs=6, space="PSUM") as psB,
        ):
            # ---- tiny consts
            eps_sb = singles.tile([P, 1], F32)
            nc.vector.memset(eps_sb, EPS)
            iota_i = singles.tile([P, 1], I32)
            nc.gpsimd.iota(iota_i, pattern=[[0, 1]], base=0, channel_multiplier=1)
            iota_f = singles.tile([P, 1], F32)
            nc.vector.tensor_copy(out=iota_f, in_=iota_i)

            # ---- input DMAs (host provides layouts that DMA at line rate)
            relidx_sb = singles.tile([1, NPC], F16)
            nc.sync.dma_start(out=relidx_sb, in_=relidx[:, :])
            xT_sb = singles.tile([P, KT, C], F16)
            H = C // 2
            for kt in range(KT):
                nc.sync.dma_start(
                    out=xT_sb[:, kt, 0:H],
                    in_=xT[P * kt:P * (kt + 1), 0:H],
                )
            wt_sb = singles.tile([P, KT, D], F16)
            nc.sync.dma_start(
                out=wt_sb, in_=wt.rearrange("(kt p) e -> p kt e", p=P)
            )
            for kt in range(KT):
                nc.sync.dma_start(
                    out=xT_sb[:, kt, H:C],
                    in_=xT[P * kt:P * (kt + 1), H:C],
                )

            if has_b:
                onesb_sb = singles.tile([1, P], F16)
                nc.vector.memset(onesb_sb, 1.0)
                bvec_sb = singles.tile([1, D], F16)
                nc.sync.dma_start(out=bvec_sb, in_=bvec[:, :])
            if not trivial_affine:
                gamma_sb = singles.tile([P, D], F32)
                nc.gpsimd.dma_start(out=gamma_sb, in_=gamma.to_broadcast((P, D)))
                beta_sb = singles.tile([P, D], F32)
                nc.gpsimd.dma_start(out=beta_sb, in_=beta.to_broadcast((P, D)))

            # normalized projection table; tile CT holds the beta row (row 0)
            nproj = singles.tile([P, CT + 1, D], BF16)
            if has_beta_row:
                nc.vector.memset(nproj[:, CT, :], 0.0)
                if not trivial_affine:
                    nc.vector.tensor_copy(out=nproj[0:1, CT, :], in_=beta_sb[0:1, :])

            # ---- build one-hot pieces on device:
            # GPSIMD broadcasts rel-indices across partitions (PE stays free
            # for matmuls), then DVE is_equal against the partition iota
            # (16-bit SBUF operands -> DVE fast mode) writes the bf16 one-hot.
            relb = singles.tile([P, NPC], F16)
            oh_sb = singles.tile([P, NPC], BF16)
            NCH = 8
            chunk = ((NPC // NCH) // P) * P
            bounds = [i * chunk for i in range(NCH)] + [NPC]
            for i in range(NCH):
                c0, c1 = bounds[i], bounds[i + 1]
                nc.gpsimd.partition_broadcast(
                    relb[:, c0:c1], relidx_sb[0:1, c0:c1]
                )

            oh_built = [False] * NCH

            def build_oh_upto(piece_end):
                col_end = piece_end * P
                for i in range(NCH):
                    if not oh_built[i] and bounds[i] < col_end:
                        c0, c1 = bounds[i], bounds[i + 1]
                        nc.vector.tensor_scalar(
                            out=oh_sb[:, c0:c1], in0=relb[:, c0:c1],
                            scalar1=iota_f, scalar2=None,
                            op0=mybir.AluOpType.is_equal,
                        )
                        oh_built[i] = True

            # ---- stage B bookkeeping: store groups emitted as soon as the
            # chunk tiles they read are normalized (overlap stores with stage A)
            by_block = [[] for _ in range(NT)]
            for p, (k, ct) in enumerate(piece_meta):
                by_block[k].append((p, ct))
            ngroups = NT // GRP
            group_need = []
            group_piece_end = []
            for g in range(ngroups):
                need = 0
                pend = 0
                for k in range(g * GRP, (g + 1) * GRP):
                    for p, ct in by_block[k]:
                        pend = max(pend, p + 1)
                        if ct < CT:
                            need = max(need, ct)
                group_need.append(need)
                group_piece_end.append(pend)

            def emit_group(g: int):
                build_oh_upto(group_piece_end[g])
                use_act = g != 3  # 7 groups on ACT, 1 on DVE (load balance)
                ot = (ostage_a if use_act else ostage_v).tile([P, GRP, D], F32)
                # tiny same-engine touch op claims the slot and absorbs the
                # DMA-release wait, keeping the big copies at <=2 sync waits
                # (walrus codegen rejects more)
                if use_act:
                    nc.scalar.copy(out=ot[0:1, 0, 0:1], in_=eps_sb[0:1, 0:1])
                else:
                    nc.vector.memset(ot[0:1, 0, 0:1], 0.0)
                for ki in range(GRP):
                    k = g * GRP + ki
                    pieces = by_block[k]
                    go = psB.tile([P, D], F32)
                    for i, (p, ct) in enumerate(pieces):
                        nc.tensor.matmul(
                            go,
                            lhsT=oh_sb[:, P * p:P * (p + 1)],
                            rhs=nproj[:, ct, :],
                            start=(i == 0),
                            stop=(i == len(pieces) - 1),
                        )
                    if use_act:
                        nc.scalar.copy(out=ot[:, ki, :], in_=go)
                    else:
                        nc.vector.tensor_copy(out=ot[:, ki, :], in_=go)
                nc.sync.dma_start(
                    out=out[P * GRP * g:P * GRP * (g + 1), :].rearrange(
                        "(c p) e -> p c e", p=P
                    ),
                    in_=ot,
                )

            # ---- stage A: proj = x @ W.T (+b) -> LayerNorm -> nproj (bf16),
            # with ready store-groups interleaved
            for ct in range(CT):
                pj = psA.tile([P, D], F32)
                for kt in range(KT):
                    nc.tensor.matmul(
                        pj,
                        lhsT=xT_sb[:, kt, P * ct:P * (ct + 1)],
                        rhs=wt_sb[:, kt, :],
                        start=(kt == 0),
                        stop=(kt == KT - 1 and not has_b),
                    )
                if has_b:
                    nc.tensor.matmul(
                        pj, lhsT=onesb_sb, rhs=bvec_sb, start=False, stop=True,
                    )

                st = stats_pool.tile([P, 6], F32, tag="bn")
                nc.vector.bn_stats(out=st, in_=pj)
                mv = stats_pool.tile([P, 2], F32, tag="mv")
                nc.vector.bn_aggr(out=mv, in_=st)
                rstd = stats_pool.tile([P, 1], F32, tag="rstd")
                nc.scalar.activation(
                    out=rstd, in_=mv[:, 1:2],
                    func=mybir.ActivationFunctionType.Sqrt,
                    bias=eps_sb, scale=1.0,
                )
                nc.vector.reciprocal(out=rstd, in_=rstd)
                if trivial_affine:
                    # normalize on ACT: out = Copy(rstd*in + (-mean*rstd))
                    negmrs = stats_pool.tile([P, 1], F32, tag="negmrs")
                    nc.vector.tensor_scalar(
                        out=negmrs, in0=mv[:, 0:1],
                        scalar1=rstd, scalar2=-1.0,
                        op0=mybir.AluOpType.mult, op1=mybir.AluOpType.mult,
                    )
                    nc.scalar.activation(
                        out=nproj[:, ct, :], in_=pj,
                        func=mybir.ActivationFunctionType.Identity,
                        bias=negmrs, scale=rstd,
                    )
                else:
                    tmp = stats_pool.tile([P, D], F32, tag="affine")
                    nc.vector.tensor_scalar(
                        out=tmp, in0=pj,
                        scalar1=mv[:, 0:1], scalar2=rstd,
                        op0=mybir.AluOpType.subtract, op1=mybir.AluOpType.mult,
                    )
                    nc.vector.tensor_mul(out=tmp, in0=tmp, in1=gamma_sb)
                    nc.vector.tensor_add(out=nproj[:, ct, :], in0=tmp, in1=beta_sb)

                for g in range(ngroups):
                    if group_need[g] == ct:
                        emit_group(g)

    nc.compile()
    return nc


# ---------------------------------------------------------------- entrypoint

def kernel(**inputs) -> np.ndarray:
    return run_dechunk(inputs)[0]


def run_dechunk(inputs, trace=False, tmpdir=None):
    x = np.asarray(inputs["hierarchical_repr"], np.float32)
    W = np.asarray(inputs["W"], np.float32)
    bvec = np.asarray(inputs["b"], np.float32)
    gamma = np.asarray(inputs["gamma"], np.float32)
    beta = np.asarray(inputs["beta"], np.float32)
    cl = np.asarray(inputs["chunk_lengths"]).astype(np.int32)
    seq_len = int(np.asarray(inputs["seq_len"]))
    assert x.shape == (B, C, D) and W.shape == (D, D) and seq_len == L

    idx = _build_indices(cl)
    piece_meta = _build_pieces(idx)
    rel = _build_relidx(idx, piece_meta)

    has_b = bool(np.any(bvec != 0))
    trivial_affine = bool(np.all(gamma == 1.0) and np.all(beta == 0.0))
    has_beta_row = bool((idx == C).any())

    nc = _build_bass(piece_meta, has_b, trivial_affine, has_beta_row)

    wt_h = np.ascontiguousarray(W.T).astype(np.float16)
    in_maps = []
    for b in range(B):
        m = {
            "xT": np.ascontiguousarray(x[b].T).astype(np.float16),
            "wt": wt_h,
            "relidx": rel[b],
        }
        if has_b:
            m["bvec"] = bvec.reshape(1, D).astype(np.float16)
        if not trivial_affine:
            m["gamma"] = gamma.reshape(1, D)
            m["beta"] = beta.reshape(1, D)
        in_maps.append(m)

    res = run_bass_kernel_spmd(
        nc, in_maps, core_ids=list(range(B)), trace=trace, tmpdir=tmpdir
    )
    out = np.stack(
        [np.asarray(res.results[i]["out"], np.float32) for i in range(B)]
    )
    return out, res


# revision 26
# speedup vs baseline: 1.7344x; 1.1063x over previous
"""DeChunkLayer Trainium2 kernel.

Computation (per batch row b):
    proj = x[b] @ W.T + b                                  # [C, D]
    nproj = LayerNorm(proj) (= (proj-mean)*rsqrt(var+eps)*gamma + beta)
    out[t] = nproj[idx[t]] for valid t, beta for padded t  # [L, D]

Key insight: LayerNorm commutes with the ragged expansion (each output row is a
copy of a chunk row, so its stats equal the chunk row's stats). So we normalize
the C=2048 chunk rows once and then expand, instead of normalizing L=8192 rows.

The ragged expansion runs on the TensorEngine as a block-sparse one-hot matmul:
for each 128-token output block, accumulate over the few 128-chunk tiles the
block draws from:  out_block = sum_ct OH[k,ct].T @ nproj[ct].  The one-hot
pieces are host-built from chunk_lengths (a tiny input), shipped as fp8 (0/1 is
exact) and cast to bf16 by the DMA engines on load. Piece->tile wiring is baked
at trace time (kernel JIT-specialized per input, SPMD-uniform across the 8
cores: the piece list is the union over cores; a core's one-hot data is zero
for pieces it doesn't use).

Sharding: data-parallel over the batch dim, one NeuronCore per row (8 cores);
W/bias/gamma/beta replicated. Padded tail positions map to a dedicated beta row
(row C) of the nproj table; fully-padded tail blocks are skipped entirely (the
runtime pre-zeros output buffers).
"""

import sys

for _p in ("/opt/trn_rl_repo",):
    if _p not in sys.path:
        sys.path.insert(0, _p)

import numpy as np
import ml_dtypes

import concourse.bass as bass
import concourse.tile as tile
from concourse import bacc, mybir
from concourse.bass_utils import run_bass_kernel_spmd

B, C, D, L = 8, 2048, 512, 8192
P = 128           # partitions
CT = C // P       # 16 chunk tiles
NT = L // P       # 64 output-token blocks
KT = D // P       # 4 contraction tiles
EPS = 1e-5
GRP = 8           # output blocks per store DMA (2 MiB batches)
NOHCH = 4         # one-hot DMA chunks

F32 = mybir.dt.float32
BF16 = mybir.dt.bfloat16
F16 = mybir.dt.float16
FP8 = mybir.dt.float8e4
NP_FP8 = mybir.dt.np(FP8)


# ----------------------------------------------------------------- host logic

def _build_indices(chunk_lengths: np.ndarray) -> np.ndarray:
    """idx[b, t] = source chunk row for output token t; C means 'beta row'."""
    cum = np.cumsum(chunk_lengths.astype(np.int64), axis=1)
    pos = np.arange(L)
    idx = np.empty((B, L), np.int32)
    for b in range(B):
        i = np.searchsorted(cum[b], pos, side="right").astype(np.int32)
        i = np.minimum(i, C - 1)
        i[pos >= cum[b, -1]] = C  # padded positions -> beta row
        idx[b] = i
    return idx


def _n_valid_blocks(chunk_lengths: np.ndarray) -> int:
    """Blocks below this index have at least one non-padded position on some
    core; blocks at/above it are all-beta on every core (skippable when the
    affine part is trivial, since output buffers are pre-zeroed)."""
    total = chunk_lengths.astype(np.int64).sum(axis=1).max()
    return min(NT, -(-int(total) // P))


def _build_pieces(idx: np.ndarray, nt_eff: int):
    """Per 128-token block, the union (over cores) of touched 128-chunk tiles."""
    blocks = idx.reshape(B, NT, P)
    piece_meta = []  # list of (block k, chunk-tile ct)
    for k in range(nt_eff):
        tiles = sorted(set((blocks[:, k, :] // P).ravel().tolist()))
        for ct in tiles:
            piece_meta.append((k, ct))
    return piece_meta


def _build_onehot(idx: np.ndarray, piece_meta) -> np.ndarray:
    """oh[b, j, p, tt] = 1 iff idx[b, 128k+tt] == 128*ct + j for piece p=(k,ct).

    Layout is partition-first so the whole tensor loads with line-rate DMAs;
    fp8 (exact for 0/1), cast to bf16 by the DMA engines.
    """
    NP = len(piece_meta)
    blocks = idx.reshape(B, NT, P)
    oh = np.zeros((B, P, NP, P), dtype=NP_FP8)
    for p, (k, ct) in enumerate(piece_meta):
        rel = blocks[:, k, :] - P * ct  # [B, 128]
        for b in range(B):
            tt = np.nonzero((rel[b] >= 0) & (rel[b] < P))[0]
            oh[b, rel[b, tt], p, tt] = 1
    return oh


# --------------------------------------------------------------- bass kernel

def _build_bass(piece_meta, nt_eff, has_b, trivial_affine, has_beta_row):
    NP = len(piece_meta)
    # Bacc (not bare Bass): its compile() legalizes drains/sync-waits into
    # forms the walrus codegen accepts.
    nc = bacc.Bacc(None)

    xT = nc.declare_dram_parameter("xT", [D, C], F16, isOutput=False)
    wt = nc.declare_dram_parameter("wt", [D, D], F16, isOutput=False)
    oh = nc.declare_dram_parameter("oh", [P, NP, P], FP8, isOutput=False)
    if has_b:
        bvec = nc.declare_dram_parameter("bvec", [1, D], F16, isOutput=False)
    if not trivial_affine:
        gamma = nc.declare_dram_parameter("gamma", [1, D], F32, isOutput=False)
        beta = nc.declare_dram_parameter("beta", [1, D], F32, isOutput=False)
    out = nc.declare_dram_parameter("out", [L, D], F32, isOutput=True)

    with tile.TileContext(nc) as tc:
        with (
            tc.tile_pool(name="singles", bufs=1) as singles,
            tc.tile_pool(name="stats", bufs=8) as stats_pool,
            tc.tile_pool(name="ostage_a", bufs=2) as ostage_a,
            tc.tile_pool(name="ostage_v", bufs=2) as ostage_v,
            tc.tile_pool(name="psA", bufs=2, space="PSUM") as psA,
            tc.tile_pool(name="psB", bufs=6, space="PSUM") as psB,
        ):
            # ---- tiny consts
            eps_sb = singles.tile([P, 1], F32)
            nc.vector.memset(eps_sb, EPS)

            # ---- input DMAs: xT first so the TensorEngine ramps early
            xT_sb = singles.tile([P, KT, C], F16)
            for kt in range(KT):
                nc.sync.dma_start(
                    out=xT_sb[:, kt, :], in_=xT[P * kt:P * (kt + 1), :]
                )
            wt_sb = singles.tile([P, KT, D], F16)
            nc.sync.dma_start(
                out=wt_sb, in_=wt.rearrange("(kt p) e -> p kt e", p=P)
            )

            if has_b:
                onesb_sb = singles.tile([1, P], F16)
                nc.vector.memset(onesb_sb, 1.0)
                bvec_sb = singles.tile([1, D], F16)
                nc.sync.dma_start(out=bvec_sb, in_=bvec[:, :])
            if not trivial_affine:
                gamma_sb = singles.tile([P, D], F32)
                nc.gpsimd.dma_start(out=gamma_sb, in_=gamma.to_broadcast((P, D)))
                beta_sb = singles.tile([P, D], F32)
                nc.gpsimd.dma_start(out=beta_sb, in_=beta.to_broadcast((P, D)))

            # normalized projection table; tile CT holds the beta row (row 0)
            nproj = singles.tile([P, CT + 1, D], BF16)
            if has_beta_row:
                nc.vector.memset(nproj[:, CT, :], 0.0)
                if not trivial_affine:
                    nc.vector.tensor_copy(out=nproj[0:1, CT, :], in_=beta_sb[0:1, :])

            # ---- one-hot pieces: fp8 in DRAM, DMA-cast to bf16, loaded in
            # chunks just before the store groups that consume them
            oh_sb = singles.tile([P, NP, P], BF16)
            chunk = -(-NP // NOHCH)
            bounds = [min(NP, i * chunk) for i in range(NOHCH)] + [NP]
            oh_loaded = [False] * NOHCH

            def load_oh_upto(piece_end):
                for i in range(NOHCH):
                    if not oh_loaded[i] and bounds[i] < piece_end:
                        c0, c1 = bounds[i], bounds[i + 1]
                        nc.gpsimd.dma_start(
                            out=oh_sb[:, c0:c1, :], in_=oh[:, c0:c1, :]
                        )
                        oh_loaded[i] = True

            # ---- stage B bookkeeping
            by_block = [[] for _ in range(nt_eff)]
            for p, (k, ct) in enumerate(piece_meta):
                by_block[k].append((p, ct))
            ngroups = -(-nt_eff // GRP)
            group_need = []
            group_piece_end = []
            for g in range(ngroups):
                need, pend = 0, 0
                for k in range(g * GRP, min((g + 1) * GRP, nt_eff)):
                    for p, ct in by_block[k]:
                        pend = max(pend, p + 1)
                        if ct < CT:
                            need = max(need, ct)
                group_need.append(need)
                group_piece_end.append(pend)

            def emit_group(g: int):
                load_oh_upto(group_piece_end[g])
                nblk = min(GRP, nt_eff - g * GRP)
                use_act = g % 2 == 0  # alternate evacuation engine per group
                ot = (ostage_a if use_act else ostage_v).tile([P, GRP, D], F32)
                # tiny same-engine touch op claims the slot and absorbs the
                # DMA-release wait, keeping the big copies at <=2 sync waits
                # (walrus codegen rejects more)
                if use_act:
                    nc.scalar.copy(out=ot[0:1, 0, 0:1], in_=eps_sb[0:1, 0:1])
                else:
                    nc.vector.memset(ot[0:1, 0, 0:1], 0.0)
                for ki in range(nblk):
                    k = g * GRP + ki
                    pieces = by_block[k]
                    go = psB.tile([P, D], F32)
                    for i, (p, ct) in enumerate(pieces):
                        nc.tensor.matmul(
                            go,
                            lhsT=oh_sb[:, p, :],
                            rhs=nproj[:, ct, :],
                            start=(i == 0),
                            stop=(i == len(pieces) - 1),
                        )
                    if use_act:
                        nc.scalar.copy(out=ot[:, ki, :], in_=go)
                    else:
                        nc.vector.tensor_copy(out=ot[:, ki, :], in_=go)
                nc.sync.dma_start(
                    out=out[P * GRP * g:P * (GRP * g + nblk), :].rearrange(
                        "(c p) e -> p c e", p=P
                    ),
                    in_=ot[:, :nblk, :],
                )

            # ---- stage A: proj = x @ W.T (+b) -> LayerNorm -> nproj (bf16),
            # with ready store-groups interleaved
            for ct in range(CT):
                pj = psA.tile([P, D], F32)
                for kt in range(KT):
                    nc.tensor.matmul(
                        pj,
                        lhsT=xT_sb[:, kt, P * ct:P * (ct + 1)],
                        rhs=wt_sb[:, kt, :],
                        start=(kt == 0),
                        stop=(kt == KT - 1 and not has_b),
                    )
                if has_b:
                    nc.tensor.matmul(
                        pj, lhsT=onesb_sb, rhs=bvec_sb, start=False, stop=True,
                    )

                st = stats_pool.tile([P, 6], F32, tag="bn")
                nc.vector.bn_stats(out=st, in_=pj)
                mv = stats_pool.tile([P, 2], F32, tag="mv")
                nc.vector.bn_aggr(out=mv, in_=st)
                rstd = stats_pool.tile([P, 1], F32, tag="rstd")
                nc.scalar.activation(
                    out=rstd, in_=mv[:, 1:2],
                    func=mybir.ActivationFunctionType.Sqrt,
                    bias=eps_sb, scale=1.0,
                )
                nc.vector.reciprocal(out=rstd, in_=rstd)
                if trivial_affine:
                    # normalize on ACT: out = Identity(rstd*in + (-mean*rstd))
                    negmrs = stats_pool.tile([P, 1], F32, tag="negmrs")
                    nc.vector.tensor_scalar(
                        out=negmrs, in0=mv[:, 0:1],
                        scalar1=rstd, scalar2=-1.0,
                        op0=mybir.AluOpType.mult, op1=mybir.AluOpType.mult,
                    )
                    nc.scalar.activation(
                        out=nproj[:, ct, :], in_=pj,
                        func=mybir.ActivationFunctionType.Identity,
                        bias=negmrs, scale=rstd,
                    )
                else:
                    tmp = stats_pool.tile([P, D], F32, tag="affine")
                    nc.vector.tensor_scalar(
                        out=tmp, in0=pj,
                        scalar1=mv[:, 0:1], scalar2=rstd,
                        op0=mybir.AluOpType.subtract, op1=mybir.AluOpType.mult,
                    )
                    nc.vector.tensor_mul(out=tmp, in0=tmp, in1=gamma_sb)
                    nc.vector.tensor_add(out=nproj[:, ct, :], in0=tmp, in1=beta_sb)

                for g in range(ngroups):
                    if group_need[g] == ct:
                        emit_group(g)

    nc.compile()
    return nc


# ---------------------------------------------------------------- entrypoint

def kernel(**inputs) -> np.ndarray:
    return run_dechunk(inputs)[0]


def run_dechunk(inputs, trace=False, tmpdir=None):
    x = np.asarray(inputs["hierarchical_repr"], np.float32)
    W = np.asarray(inputs["W"], np.float32)
    bvec = np.asarray(inputs["b"], np.float32)
    gamma = np.asarray(inputs["gamma"], np.float32)
    beta = np.asarray(inputs["beta"], np.float32)
    cl = np.asarray(inputs["chunk_lengths"]).astype(np.int32)
    seq_len = int(np.asarray(inputs["seq_len"]))
    assert x.shape == (B, C, D) and W.shape == (D, D) and seq_len == L

    idx = _build_indices(cl)
    has_b = bool(np.any(bvec != 0))
    trivial_affine = bool(np.all(gamma == 1.0) and np.all(beta == 0.0))
    # padded positions produce exactly `beta`; when beta==0 the pre-zeroed
    # output buffer already holds that, so all-padded blocks can be skipped
    nt_eff = _n_valid_blocks(cl) if trivial_affine else NT
    piece_meta = _build_pieces(idx, nt_eff)
    oh = _build_onehot(idx, piece_meta)
    has_beta_row = bool((idx[:, :nt_eff * P] == C).any())

    nc = _build_bass(piece_meta, nt_eff, has_b, trivial_affine, has_beta_row)

    wt_h = np.ascontiguousarray(W.T).astype(np.float16)
    in_maps = []
    for b in range(B):
        m = {
            "xT": np.ascontiguousarray(x[b].T).astype(np.float16),
            "wt": wt_h,
            "oh": oh[b],
        }
        if has_b:
            m["bvec"] = bvec.reshape(1, D).astype(np.float16)
        if not trivial_affine:
            m["gamma"] = gamma.reshape(1, D)
            m["beta"] = beta.reshape(1, D)
        in_maps.append(m)

    res = run_bass_kernel_spmd(
        nc, in_maps, core_ids=list(range(B)), trace=trace, tmpdir=tmpdir
    )
    out = np.stack(
        [np.asarray(res.results[i]["out"], np.float32) for i in range(B)]
    )
    return out, res


# revision 27
# speedup vs baseline: 1.7969x; 1.0361x over previous
"""DeChunkLayer Trainium2 kernel.

Computation (per batch row b):
    proj = x[b] @ W.T + b                                  # [C, D]
    nproj = LayerNorm(proj) (= (proj-mean)*rsqrt(var+eps)*gamma + beta)
    out[t] = nproj[idx[t]] for valid t, beta for padded t  # [L, D]

Key insight: LayerNorm commutes with the ragged expansion (each output row is a
copy of a chunk row, so its stats equal the chunk row's stats). So we normalize
the C=2048 chunk rows once and then expand, instead of normalizing L=8192 rows.

The ragged expansion runs on the TensorEngine as a block-sparse one-hot matmul:
for each 128-token output block, accumulate over the few 128-chunk tiles the
block draws from:  out_block = sum_ct OH[k,ct].T @ nproj[ct].  The one-hot
pieces are host-built from chunk_lengths (a tiny input), shipped as fp8 (0/1 is
exact) and cast to bf16 by the DMA engines on load. Piece->tile wiring is baked
at trace time (kernel JIT-specialized per input, SPMD-uniform across the 8
cores: the piece list is the union over cores; a core's one-hot data is zero
for pieces it doesn't use).

Sharding: data-parallel over the batch dim, one NeuronCore per row (8 cores);
W/bias/gamma/beta replicated. Padded tail positions map to a dedicated beta row
(row C) of the nproj table; fully-padded tail blocks are skipped entirely (the
runtime pre-zeros output buffers).
"""

import sys

for _p in ("/opt/trn_rl_repo",):
    if _p not in sys.path:
        sys.path.insert(0, _p)

import numpy as np
import ml_dtypes

import concourse.bass as bass
import concourse.tile as tile
from concourse import bacc, mybir
from concourse.bass_utils import run_bass_kernel_spmd

B, C, D, L = 8, 2048, 512, 8192
P = 128           # partitions
CT = C // P       # 16 chunk tiles
NT = L // P       # 64 output-token blocks
KT = D // P       # 4 contraction tiles
EPS = 1e-5
GRP = 8           # output blocks per store DMA (2 MiB batches)
NOHCH = 4         # one-hot DMA chunks

F32 = mybir.dt.float32
BF16 = mybir.dt.bfloat16
F16 = mybir.dt.float16
FP8 = mybir.dt.float8e4
NP_FP8 = mybir.dt.np(FP8)


# ----------------------------------------------------------------- host logic

def _build_indices(chunk_lengths: np.ndarray) -> np.ndarray:
    """idx[b, t] = source chunk row for output token t; C means 'beta row'."""
    cum = np.cumsum(chunk_lengths.astype(np.int64), axis=1)
    pos = np.arange(L)
    idx = np.empty((B, L), np.int32)
    for b in range(B):
        i = np.searchsorted(cum[b], pos, side="right").astype(np.int32)
        i = np.minimum(i, C - 1)
        i[pos >= cum[b, -1]] = C  # padded positions -> beta row
        idx[b] = i
    return idx


def _n_valid_blocks(chunk_lengths: np.ndarray) -> int:
    """Blocks below this index have at least one non-padded position on some
    core; blocks at/above it are all-beta on every core (skippable when the
    affine part is trivial, since output buffers are pre-zeroed)."""
    total = chunk_lengths.astype(np.int64).sum(axis=1).max()
    return min(NT, -(-int(total) // P))


def _build_pieces(idx: np.ndarray, nt_eff: int):
    """Per 128-token block, the union (over cores) of touched 128-chunk tiles."""
    blocks = idx.reshape(B, NT, P)
    piece_meta = []  # list of (block k, chunk-tile ct)
    for k in range(nt_eff):
        tiles = sorted(set((blocks[:, k, :] // P).ravel().tolist()))
        for ct in tiles:
            piece_meta.append((k, ct))
    return piece_meta


def _build_onehot(idx: np.ndarray, piece_meta) -> np.ndarray:
    """oh[b, j, p, tt] = 1 iff idx[b, 128k+tt] == 128*ct + j for piece p=(k,ct).

    Layout is partition-first so the whole tensor loads with line-rate DMAs;
    fp8 (exact for 0/1), cast to bf16 by the DMA engines.
    """
    NP = len(piece_meta)
    blocks = idx.reshape(B, NT, P)
    oh = np.zeros((B, P, NP, P), dtype=NP_FP8)
    for p, (k, ct) in enumerate(piece_meta):
        rel = blocks[:, k, :] - P * ct  # [B, 128]
        for b in range(B):
            tt = np.nonzero((rel[b] >= 0) & (rel[b] < P))[0]
            oh[b, rel[b, tt], p, tt] = 1
    return oh


# --------------------------------------------------------------- bass kernel

def _build_bass(piece_meta, nt_eff, has_b, trivial_affine, has_beta_row):
    NP = len(piece_meta)
    # Bacc (not bare Bass): its compile() legalizes drains/sync-waits into
    # forms the walrus codegen accepts.
    nc = bacc.Bacc(None)

    # x.T and W.T concatenated host-side: one line-rate DMA starts the PE fast
    xw = nc.declare_dram_parameter("xw", [D, C + D], F16, isOutput=False)
    oh = nc.declare_dram_parameter("oh", [P, NP, P], FP8, isOutput=False)
    if has_b:
        bvec = nc.declare_dram_parameter("bvec", [1, D], F16, isOutput=False)
    if not trivial_affine:
        gamma = nc.declare_dram_parameter("gamma", [1, D], F32, isOutput=False)
        beta = nc.declare_dram_parameter("beta", [1, D], F32, isOutput=False)
    out = nc.declare_dram_parameter("out", [L, D], F32, isOutput=True)

    with tile.TileContext(nc) as tc:
        with (
            tc.tile_pool(name="singles", bufs=1) as singles,
            tc.tile_pool(name="stats", bufs=8) as stats_pool,
            tc.tile_pool(name="ostage_a", bufs=2) as ostage_a,
            tc.tile_pool(name="ostage_v", bufs=2) as ostage_v,
            tc.tile_pool(name="psA", bufs=2, space="PSUM") as psA,
            tc.tile_pool(name="psB", bufs=6, space="PSUM") as psB,
        ):
            # ---- tiny consts
            eps_sb = singles.tile([P, 1], F32)
            nc.vector.memset(eps_sb, EPS)

            # ---- input DMA: single transfer so the TensorEngine ramps early
            xw_sb = singles.tile([P, KT, C + D], F16)
            nc.sync.dma_start(
                out=xw_sb, in_=xw.rearrange("(kt p) m -> p kt m", p=P)
            )

            if has_b:
                onesb_sb = singles.tile([1, P], F16)
                nc.vector.memset(onesb_sb, 1.0)
                bvec_sb = singles.tile([1, D], F16)
                nc.sync.dma_start(out=bvec_sb, in_=bvec[:, :])
            if not trivial_affine:
                gamma_sb = singles.tile([P, D], F32)
                nc.gpsimd.dma_start(out=gamma_sb, in_=gamma.to_broadcast((P, D)))
                beta_sb = singles.tile([P, D], F32)
                nc.gpsimd.dma_start(out=beta_sb, in_=beta.to_broadcast((P, D)))

            # normalized projection table; tile CT holds the beta row (row 0)
            nproj = singles.tile([P, CT + 1, D], BF16)
            if has_beta_row:
                nc.vector.memset(nproj[:, CT, :], 0.0)
                if not trivial_affine:
                    nc.vector.tensor_copy(out=nproj[0:1, CT, :], in_=beta_sb[0:1, :])

            # ---- one-hot pieces: fp8 in DRAM, DMA-cast to bf16, loaded in
            # chunks just before the store groups that consume them
            oh_sb = singles.tile([P, NP, P], BF16)
            chunk = -(-NP // NOHCH)
            bounds = [min(NP, i * chunk) for i in range(NOHCH)] + [NP]
            oh_loaded = [False] * NOHCH

            def load_oh_upto(piece_end):
                for i in range(NOHCH):
                    if not oh_loaded[i] and bounds[i] < piece_end:
                        c0, c1 = bounds[i], bounds[i + 1]
                        nc.gpsimd.dma_start(
                            out=oh_sb[:, c0:c1, :], in_=oh[:, c0:c1, :]
                        )
                        oh_loaded[i] = True

            # ---- stage B bookkeeping
            by_block = [[] for _ in range(nt_eff)]
            for p, (k, ct) in enumerate(piece_meta):
                by_block[k].append((p, ct))
            ngroups = -(-nt_eff // GRP)
            group_need = []
            group_piece_end = []
            for g in range(ngroups):
                need, pend = 0, 0
                for k in range(g * GRP, min((g + 1) * GRP, nt_eff)):
                    for p, ct in by_block[k]:
                        pend = max(pend, p + 1)
                        if ct < CT:
                            need = max(need, ct)
                group_need.append(need)
                group_piece_end.append(pend)

            def emit_group(g: int):
                load_oh_upto(group_piece_end[g])
                nblk = min(GRP, nt_eff - g * GRP)
                use_act = g % 2 == 0  # alternate evacuation engine per group
                ot = (ostage_a if use_act else ostage_v).tile([P, GRP, D], F32)
                # tiny same-engine touch op claims the slot and absorbs the
                # DMA-release wait, keeping the big copies at <=2 sync waits
                # (walrus codegen rejects more)
                if use_act:
                    nc.scalar.copy(out=ot[0:1, 0, 0:1], in_=eps_sb[0:1, 0:1])
                else:
                    nc.vector.memset(ot[0:1, 0, 0:1], 0.0)
                for ki in range(nblk):
                    k = g * GRP + ki
                    pieces = by_block[k]
                    go = psB.tile([P, D], F32)
                    for i, (p, ct) in enumerate(pieces):
                        nc.tensor.matmul(
                            go,
                            lhsT=oh_sb[:, p, :],
                            rhs=nproj[:, ct, :],
                            start=(i == 0),
                            stop=(i == len(pieces) - 1),
                        )
                    if use_act:
                        nc.scalar.copy(out=ot[:, ki, :], in_=go)
                    else:
                        nc.vector.tensor_copy(out=ot[:, ki, :], in_=go)
                nc.sync.dma_start(
                    out=out[P * GRP * g:P * (GRP * g + nblk), :].rearrange(
                        "(c p) e -> p c e", p=P
                    ),
                    in_=ot[:, :nblk, :],
                )

            # ---- stage A: proj = x @ W.T (+b) -> LayerNorm -> nproj (bf16),
            # with ready store-groups interleaved
            for ct in range(CT):
                pj = psA.tile([P, D], F32)
                for kt in range(KT):
                    nc.tensor.matmul(
                        pj,
                        lhsT=xw_sb[:, kt, P * ct:P * (ct + 1)],
                        rhs=xw_sb[:, kt, C:C + D],
                        start=(kt == 0),
                        stop=(kt == KT - 1 and not has_b),
                    )
                if has_b:
                    nc.tensor.matmul(
                        pj, lhsT=onesb_sb, rhs=bvec_sb, start=False, stop=True,
                    )

                st = stats_pool.tile([P, 6], F32, tag="bn")
                nc.vector.bn_stats(out=st, in_=pj)
                mv = stats_pool.tile([P, 2], F32, tag="mv")
                nc.vector.bn_aggr(out=mv, in_=st)
                rstd = stats_pool.tile([P, 1], F32, tag="rstd")
                nc.scalar.activation(
                    out=rstd, in_=mv[:, 1:2],
                    func=mybir.ActivationFunctionType.Sqrt,
                    bias=eps_sb, scale=1.0,
                )
                nc.vector.reciprocal(out=rstd, in_=rstd)
                if trivial_affine:
                    # normalize on ACT: out = Identity(rstd*in + (-mean*rstd))
                    negmrs = stats_pool.tile([P, 1], F32, tag="negmrs")
                    nc.vector.tensor_scalar(
                        out=negmrs, in0=mv[:, 0:1],
                        scalar1=rstd, scalar2=-1.0,
                        op0=mybir.AluOpType.mult, op1=mybir.AluOpType.mult,
                    )
                    nc.scalar.activation(
                        out=nproj[:, ct, :], in_=pj,
                        func=mybir.ActivationFunctionType.Identity,
                        bias=negmrs, scale=rstd,
                    )
                else:
                    tmp = stats_pool.tile([P, D], F32, tag="affine")
                    nc.vector.tensor_scalar(
                        out=tmp, in0=pj,
                        scalar1=mv[:, 0:1], scalar2=rstd,
                        op0=mybir.AluOpType.subtract, op1=mybir.AluOpType.mult,
                    )
                    nc.vector.tensor_mul(out=tmp, in0=tmp, in1=gamma_sb)
                    nc.vector.tensor_add(out=nproj[:, ct, :], in0=tmp, in1=beta_sb)

                for g in range(ngroups):
                    if group_need[g] == ct:
                        emit_group(g)

    nc.compile()
    return nc


# ---------------------------------------------------------------- entrypoint

def kernel(**inputs) -> np.ndarray:
    return run_dechunk(inputs)[0]


def run_dechunk(inputs, trace=False, tmpdir=None):
    x = np.asarray(inputs["hierarchical_repr"], np.float32)
    W = np.asarray(inputs["W"], np.float32)
    bvec = np.asarray(inputs["b"], np.float32)
    gamma = np.asarray(inputs["gamma"], np.float32)
    beta = np.asarray(inputs["beta"], np.float32)
    cl = np.asarray(inputs["chunk_lengths"]).astype(np.int32)
    seq_len = int(np.asarray(inputs["seq_len"]))
    assert x.shape == (B, C, D) and W.shape == (D, D) and seq_len == L

    idx = _build_indices(cl)
    has_b = bool(np.any(bvec != 0))
    trivial_affine = bool(np.all(gamma == 1.0) and np.all(beta == 0.0))
    # padded positions produce exactly `beta`; when beta==0 the pre-zeroed
    # output buffer already holds that, so all-padded blocks can be skipped
    nt_eff = _n_valid_blocks(cl) if trivial_affine else NT
    piece_meta = _build_pieces(idx, nt_eff)
    oh = _build_onehot(idx, piece_meta)
    has_beta_row = bool((idx[:, :nt_eff * P] == C).any())

    nc = _build_bass(piece_meta, nt_eff, has_b, trivial_affine, has_beta_row)

    wt_h = W.T.astype(np.float16)
    in_maps = []
    for b in range(B):
        m = {
            "xw": np.ascontiguousarray(
                np.concatenate([x[b].T.astype(np.float16), wt_h], axis=1)
            ),
            "oh": oh[b],
        }
        if has_b:
            m["bvec"] = bvec.reshape(1, D).astype(np.float16)
        if not trivial_affine:
            m["gamma"] = gamma.reshape(1, D)
            m["beta"] = beta.reshape(1, D)
        in_maps.append(m)

    res = run_bass_kernel_spmd(
        nc, in_maps, core_ids=list(range(B)), trace=trace, tmpdir=tmpdir
    )
    out = np.stack(
        [np.asarray(res.results[i]["out"], np.float32) for i in range(B)]
    )
    return out, res


# revision 28
# speedup vs baseline: 1.9001x; 1.0574x over previous
"""DeChunkLayer Trainium2 kernel.

Computation (per batch row b):
    proj = x[b] @ W.T + b                                  # [C, D]
    nproj = LayerNorm(proj) (= (proj-mean)*rsqrt(var+eps)*gamma + beta)
    out[t] = nproj[idx[t]] for valid t, beta for padded t  # [L, D]

Key insight: LayerNorm commutes with the ragged expansion (each output row is a
copy of a chunk row, so its stats equal the chunk row's stats). So we normalize
the C=2048 chunk rows once and then expand, instead of normalizing L=8192 rows.

The ragged expansion runs on the TensorEngine as a block-sparse one-hot matmul:
for each 128-token output block, accumulate over the few 128-chunk tiles the
block draws from:  out_block = sum_ct OH[k,ct].T @ nproj[ct].  The one-hot
pieces are host-built from chunk_lengths (a tiny input), shipped as fp8 (0/1 is
exact) and cast to bf16 by the DMA engines on load. Piece->tile wiring is baked
at trace time (kernel JIT-specialized per input, SPMD-uniform across the 8
cores: the piece list is the union over cores; a core's one-hot data is zero
for pieces it doesn't use).

Sharding: data-parallel over the batch dim, one NeuronCore per row (8 cores);
W/bias/gamma/beta replicated. Padded tail positions map to a dedicated beta row
(row C) of the nproj table; fully-padded tail blocks are skipped entirely (the
runtime pre-zeros output buffers).
"""

import sys

for _p in ("/opt/trn_rl_repo",):
    if _p not in sys.path:
        sys.path.insert(0, _p)

import numpy as np
import ml_dtypes

import concourse.bass as bass
import concourse.tile as tile
from concourse import bacc, mybir
from concourse.bass_utils import run_bass_kernel_spmd

B, C, D, L = 8, 2048, 512, 8192
P = 128           # partitions
CT = C // P       # 16 chunk tiles
NT = L // P       # 64 output-token blocks
KT = D // P       # 4 contraction tiles
EPS = 1e-5
GRP = 8           # output blocks per store DMA (2 MiB batches)
NOHCH = 4         # one-hot DMA chunks

F32 = mybir.dt.float32
BF16 = mybir.dt.bfloat16
F16 = mybir.dt.float16
FP8 = mybir.dt.float8e4
NP_FP8 = mybir.dt.np(FP8)


# ----------------------------------------------------------------- host logic

def _build_indices(chunk_lengths: np.ndarray) -> np.ndarray:
    """idx[b, t] = source chunk row for output token t; C means 'beta row'."""
    cum = np.cumsum(chunk_lengths.astype(np.int64), axis=1)
    pos = np.arange(L)
    idx = np.empty((B, L), np.int32)
    for b in range(B):
        i = np.searchsorted(cum[b], pos, side="right").astype(np.int32)
        i = np.minimum(i, C - 1)
        i[pos >= cum[b, -1]] = C  # padded positions -> beta row
        idx[b] = i
    return idx


def _n_valid_blocks(chunk_lengths: np.ndarray) -> int:
    """Blocks below this index have at least one non-padded position on some
    core; blocks at/above it are all-beta on every core (skippable when the
    affine part is trivial, since output buffers are pre-zeroed)."""
    total = chunk_lengths.astype(np.int64).sum(axis=1).max()
    return min(NT, -(-int(total) // P))


def _build_pieces(idx: np.ndarray, nt_eff: int):
    """Per 128-token block, the union (over cores) of touched 128-chunk tiles."""
    blocks = idx.reshape(B, NT, P)
    piece_meta = []  # list of (block k, chunk-tile ct)
    for k in range(nt_eff):
        tiles = sorted(set((blocks[:, k, :] // P).ravel().tolist()))
        for ct in tiles:
            piece_meta.append((k, ct))
    return piece_meta


def _build_onehot(idx: np.ndarray, piece_meta) -> np.ndarray:
    """oh[b, j, p, tt] = 1 iff idx[b, 128k+tt] == 128*ct + j for piece p=(k,ct).

    Layout is partition-first so the whole tensor loads with line-rate DMAs.
    """
    NP = len(piece_meta)
    blocks = idx.reshape(B, NT, P)
    oh = np.zeros((B, P, NP, P), dtype=ml_dtypes.bfloat16)
    for p, (k, ct) in enumerate(piece_meta):
        rel = blocks[:, k, :] - P * ct  # [B, 128]
        for b in range(B):
            tt = np.nonzero((rel[b] >= 0) & (rel[b] < P))[0]
            oh[b, rel[b, tt], p, tt] = 1
    return oh


# --------------------------------------------------------------- bass kernel

def _build_bass(piece_meta, nt_eff, has_b, trivial_affine, has_beta_row):
    NP = len(piece_meta)
    # Bacc (not bare Bass): its compile() legalizes drains/sync-waits into
    # forms the walrus codegen accepts.
    nc = bacc.Bacc(None)

    # x.T and W.T concatenated host-side: one line-rate DMA starts the PE fast
    xw = nc.declare_dram_parameter("xw", [D, C + D], F16, isOutput=False)
    oh = nc.declare_dram_parameter("oh", [P, NP, P], BF16, isOutput=False)
    if has_b:
        bvec = nc.declare_dram_parameter("bvec", [1, D], F16, isOutput=False)
    if not trivial_affine:
        gamma = nc.declare_dram_parameter("gamma", [1, D], F32, isOutput=False)
        beta = nc.declare_dram_parameter("beta", [1, D], F32, isOutput=False)
    out = nc.declare_dram_parameter("out", [L, D], F32, isOutput=True)

    with tile.TileContext(nc) as tc:
        with (
            tc.tile_pool(name="singles", bufs=1) as singles,
            tc.tile_pool(name="stats", bufs=8) as stats_pool,
            tc.tile_pool(name="ostage_a", bufs=2) as ostage_a,
            tc.tile_pool(name="ostage_v", bufs=2) as ostage_v,
            tc.tile_pool(name="psA", bufs=2, space="PSUM") as psA,
            tc.tile_pool(name="psB", bufs=6, space="PSUM") as psB,
        ):
            # ---- tiny consts
            eps_sb = singles.tile([P, 1], F32)
            nc.vector.memset(eps_sb, EPS)

            # ---- input DMA: single transfer so the TensorEngine ramps early
            xw_sb = singles.tile([P, KT, C + D], F16)
            nc.sync.dma_start(
                out=xw_sb, in_=xw.rearrange("(kt p) m -> p kt m", p=P)
            )

            if has_b:
                onesb_sb = singles.tile([1, P], F16)
                nc.vector.memset(onesb_sb, 1.0)
                bvec_sb = singles.tile([1, D], F16)
                nc.sync.dma_start(out=bvec_sb, in_=bvec[:, :])
            if not trivial_affine:
                gamma_sb = singles.tile([P, D], F32)
                nc.gpsimd.dma_start(out=gamma_sb, in_=gamma.to_broadcast((P, D)))
                beta_sb = singles.tile([P, D], F32)
                nc.gpsimd.dma_start(out=beta_sb, in_=beta.to_broadcast((P, D)))

            # normalized projection table; tile CT holds the beta row (row 0)
            nproj = singles.tile([P, CT + 1, D], BF16)
            if has_beta_row:
                nc.vector.memset(nproj[:, CT, :], 0.0)
                if not trivial_affine:
                    nc.vector.tensor_copy(out=nproj[0:1, CT, :], in_=beta_sb[0:1, :])

            # ---- one-hot pieces: fp8 in DRAM, DMA-cast to bf16, loaded in
            # chunks just before the store groups that consume them
            oh_sb = singles.tile([P, NP, P], BF16)
            chunk = -(-NP // NOHCH)
            bounds = [min(NP, i * chunk) for i in range(NOHCH)] + [NP]
            oh_loaded = [False] * NOHCH

            def load_oh_upto(piece_end):
                for i in range(NOHCH):
                    if not oh_loaded[i] and bounds[i] < piece_end:
                        c0, c1 = bounds[i], bounds[i + 1]
                        nc.sync.dma_start(
                            out=oh_sb[:, c0:c1, :], in_=oh[:, c0:c1, :]
                        )
                        oh_loaded[i] = True

            # ---- stage B bookkeeping
            by_block = [[] for _ in range(nt_eff)]
            for p, (k, ct) in enumerate(piece_meta):
                by_block[k].append((p, ct))
            ngroups = -(-nt_eff // GRP)
            group_need = []
            group_piece_end = []
            for g in range(ngroups):
                need, pend = 0, 0
                for k in range(g * GRP, min((g + 1) * GRP, nt_eff)):
                    for p, ct in by_block[k]:
                        pend = max(pend, p + 1)
                        if ct < CT:
                            need = max(need, ct)
                group_need.append(need)
                group_piece_end.append(pend)

            def emit_group(g: int):
                load_oh_upto(group_piece_end[g])
                nblk = min(GRP, nt_eff - g * GRP)
                use_act = g % 2 == 0  # alternate evacuation engine per group
                ot = (ostage_a if use_act else ostage_v).tile([P, GRP, D], F32)
                # tiny same-engine touch op claims the slot and absorbs the
                # DMA-release wait, keeping the big copies at <=2 sync waits
                # (walrus codegen rejects more)
                if use_act:
                    nc.scalar.copy(out=ot[0:1, 0, 0:1], in_=eps_sb[0:1, 0:1])
                else:
                    nc.vector.memset(ot[0:1, 0, 0:1], 0.0)
                for ki in range(nblk):
                    k = g * GRP + ki
                    pieces = by_block[k]
                    go = psB.tile([P, D], F32)
                    for i, (p, ct) in enumerate(pieces):
                        nc.tensor.matmul(
                            go,
                            lhsT=oh_sb[:, p, :],
                            rhs=nproj[:, ct, :],
                            start=(i == 0),
                            stop=(i == len(pieces) - 1),
                        )
                    if use_act:
                        nc.scalar.copy(out=ot[:, ki, :], in_=go)
                    else:
                        nc.vector.tensor_copy(out=ot[:, ki, :], in_=go)
                nc.sync.dma_start(
                    out=out[P * GRP * g:P * (GRP * g + nblk), :].rearrange(
                        "(c p) e -> p c e", p=P
                    ),
                    in_=ot[:, :nblk, :],
                )

            # ---- stage A: proj = x @ W.T (+b) -> LayerNorm -> nproj (bf16),
            # with ready store-groups interleaved
            for ct in range(CT):
                pj = psA.tile([P, D], F32)
                for kt in range(KT):
                    nc.tensor.matmul(
                        pj,
                        lhsT=xw_sb[:, kt, P * ct:P * (ct + 1)],
                        rhs=xw_sb[:, kt, C:C + D],
                        start=(kt == 0),
                        stop=(kt == KT - 1 and not has_b),
                    )
                if has_b:
                    nc.tensor.matmul(
                        pj, lhsT=onesb_sb, rhs=bvec_sb, start=False, stop=True,
                    )

                st = stats_pool.tile([P, 6], F32, tag="bn")
                nc.vector.bn_stats(out=st, in_=pj)
                mv = stats_pool.tile([P, 2], F32, tag="mv")
                nc.vector.bn_aggr(out=mv, in_=st)
                rstd = stats_pool.tile([P, 1], F32, tag="rstd")
                nc.scalar.activation(
                    out=rstd, in_=mv[:, 1:2],
                    func=mybir.ActivationFunctionType.Sqrt,
                    bias=eps_sb, scale=1.0,
                )
                nc.vector.reciprocal(out=rstd, in_=rstd)
                if trivial_affine:
                    # normalize on ACT: out = Identity(rstd*in + (-mean*rstd))
                    negmrs = stats_pool.tile([P, 1], F32, tag="negmrs")
                    nc.vector.tensor_scalar(
                        out=negmrs, in0=mv[:, 0:1],
                        scalar1=rstd, scalar2=-1.0,
                        op0=mybir.AluOpType.mult, op1=mybir.AluOpType.mult,
                    )
                    nc.scalar.activation(
                        out=nproj[:, ct, :], in_=pj,
                        func=mybir.ActivationFunctionType.Identity,
                        bias=negmrs, scale=rstd,
                    )
                else:
                    tmp = stats_pool.tile([P, D], F32, tag="affine")
                    nc.vector.tensor_scalar(
                        out=tmp, in0=pj,
                        scalar1=mv[:, 0:1], scalar2=rstd,
                        op0=mybir.AluOpType.subtract, op1=mybir.AluOpType.mult,
                    )
                    nc.vector.tensor_mul(out=tmp, in0=tmp, in1=gamma_sb)
                    nc.vector.tensor_add(out=nproj[:, ct, :], in0=tmp, in1=beta_sb)

                for g in range(ngroups):
                    if group_need[g] == ct:
                        emit_group(g)

    nc.compile()
    return nc


# ---------------------------------------------------------------- entrypoint

def kernel(**inputs) -> np.ndarray:
    return run_dechunk(inputs)[0]


def run_dechunk(inputs, trace=False, tmpdir=None):
    x = np.asarray(inputs["hierarchical_repr"], np.float32)
    W = np.asarray(inputs["W"], np.float32)
    bvec = np.asarray(inputs["b"], np.float32)
    gamma = np.asarray(inputs["gamma"], np.float32)
    beta = np.asarray(inputs["beta"], np.float32)
    cl = np.asarray(inputs["chunk_lengths"]).astype(np.int32)
    seq_len = int(np.asarray(inputs["seq_len"]))
    assert x.shape == (B, C, D) and W.shape == (D, D) and seq_len == L

    idx = _build_indices(cl)
    has_b = bool(np.any(bvec != 0))
    trivial_affine = bool(np.all(gamma == 1.0) and np.all(beta == 0.0))
    # padded positions produce exactly `beta`; when beta==0 the pre-zeroed
    # output buffer already holds that, so all-padded blocks can be skipped
    nt_eff = _n_valid_blocks(cl) if trivial_affine else NT
    piece_meta = _build_pieces(idx, nt_eff)
    oh = _build_onehot(idx, piece_meta)
    has_beta_row = bool((idx[:, :nt_eff * P] == C).any())

    nc = _build_bass(piece_meta, nt_eff, has_b, trivial_affine, has_beta_row)

    wt_h = W.T.astype(np.float16)
    in_maps = []
    for b in range(B):
        m = {
            "xw": np.ascontiguousarray(
                np.concatenate([x[b].T.astype(np.float16), wt_h], axis=1)
            ),
            "oh": oh[b],
        }
        if has_b:
            m["bvec"] = bvec.reshape(1, D).astype(np.float16)
        if not trivial_affine:
            m["gamma"] = gamma.reshape(1, D)
            m["beta"] = beta.reshape(1, D)
        in_maps.append(m)

    res = run_bass_kernel_spmd(
        nc, in_maps, core_ids=list(range(B)), trace=trace, tmpdir=tmpdir
    )
    out = np.stack(
        [np.asarray(res.results[i]["out"], np.float32) for i in range(B)]
    )
    return out, res
